# revision 16
# baseline (speedup 1.0000x reference)
"""Trainium2 Bass kernel for nn_CrossAttentionFusion.

Problem (hardcoded shapes): B=2, C1=64, C2=256, D=256, NH=8, HD=32, H=W=64,
n = H*W = 4096 tokens per batch image.

    xl = F_lidar tokens (B, n, C1); xc = F_cam tokens (B, n, C2)
    Q = xl@Wq^T, K = xc@Wk^T, V = xc@Wv^T  (per-head HD=32)
    attn = softmax(QK^T/sqrt(HD)); out = attn@V
    x = LN1(xl@Wres^T + out@Wo^T); x = LN2(x + FFN(x)); return (B, D, H, W)

Sharding: 8 cores, zero collectives. Core i handles batch b=i//4 and the
1024-token q-slice (i%4). K/V for the whole image are recomputed per core.

Design: the kernel is ACT-bound (33.5M softmax exps per core = ~270us on the
scalar engine at 1 elem/lane/cycle). Everything else is organized to hide
under that floor:
  - Scores K^T@Q in fp16 with 4-head row-packing (tile_position); each head's
    [128,512] score block lands in its own PSUM bank (4 banks total).
  - exp runs in two [128,1024] halves (heads 01 / heads 23); while ACT exps
    one half, the PE refills the other half with the next kc's scores
    (half-quad double buffering inside 4 banks).
  - AV and the all-ones denominator matmuls are fp16 4-head column-packed,
    accumulating over kc into one PSUM bank each. Denominator rows align
    with AV head rows, so normalization is one DVE divide per (hg,qc).
  - Four sweeps (hg x qc). Projections (K/V/Q/resid) are injected into sweep
    1's PE slack; Wo+LN1+FFN for q-half 0 inject into sweeps 3/4; only the
    last q-half's epilogue is exposed.
  - LN rstd = Exp(-0.5*Log(var+eps)): Log+Exp live in one ACT table set
    (natural_log_exp_and_others), so no 2.7us table reloads interleave with
    the softmax exps. A dummy Log at kernel start pins that set.
"""

from collections import deque

import numpy as np

B, C1, C2, D, NH, H, W = 2, 64, 256, 256, 8, 64, 64
HD = D // NH                 # 32
N_TOK = H * W                # 4096 tokens per image
N_CORES = 8
CORES_PER_B = N_CORES // B   # 4
NQ = N_TOK // CORES_PER_B    # 1024 q tokens per core
EPS = 1e-5
SCALE = HD ** -0.5
KC = N_TOK // 128            # 32 k-chunks
QT_TILES = NQ // 128         # 8 q-tiles of 128
F1 = 4 * D                   # 1024 FFN hidden

_built = None


def _build():
    from contextlib import ExitStack

    import concourse.mybir as mybir
    import concourse.tile as tile
    from concourse import bacc
    from concourse.masks import make_identity

    F32 = mybir.dt.float32
    F32R = mybir.dt.float32r
    F16 = mybir.dt.float16
    AF = mybir.ActivationFunctionType
    OP = mybir.AluOpType

    nc = bacc.Bacc(trn_type="TRN2", target_bir_lowering=False, debug=False,
                   num_devices=N_CORES)

    # ---- DRAM I/O ----
    xq = nc.dram_tensor("xq", [C1, NQ], F32R, kind="ExternalInput").ap()
    xc = nc.dram_tensor("xc", [C2, N_TOK], F32R, kind="ExternalInput").ap()
    wkt = nc.dram_tensor("wkt", [C2, D], F32R, kind="ExternalInput").ap()
    wvt = nc.dram_tensor("wvt", [C2, D], F32R, kind="ExternalInput").ap()
    wqt = nc.dram_tensor("wqt", [C1, D], F32R, kind="ExternalInput").ap()
    wrt = nc.dram_tensor("wrt", [C1, D], F32R, kind="ExternalInput").ap()
    # Wo^T rows permuted+zero-padded to match the sparse attn PSUM layout
    # (4 blocks of 128 virtual-d rows; see _make_in_maps)
    wot = nc.dram_tensor("wot", [4, 128, D], F32R, kind="ExternalInput").ap()
    w1t = nc.dram_tensor("w1t", [D, F1], F32R, kind="ExternalInput").ap()
    w2t = nc.dram_tensor("w2t", [F1, D], F16, kind="ExternalInput").ap()
    g1 = nc.dram_tensor("g1", [D], F32, kind="ExternalInput").ap()
    b1 = nc.dram_tensor("b1", [D], F32, kind="ExternalInput").ap()
    g2 = nc.dram_tensor("g2", [D], F32, kind="ExternalInput").ap()
    b2 = nc.dram_tensor("b2", [D], F32, kind="ExternalInput").ap()
    bf1 = nc.dram_tensor("bf1", [F1], F32, kind="ExternalInput").ap()
    bf2 = nc.dram_tensor("bf2", [D], F32, kind="ExternalInput").ap()
    out = nc.dram_tensor("out", [NQ, D], F32, kind="ExternalOutput").ap()

    with tile.TileContext(nc) as tc, ExitStack() as ctx:
        # ---- PSUM pools (8 banks total) ----
        # sc: 4 banks (scores, one per head of the active group)
        # avp: 2 banks (attn-V accumulator + ones/denominator accumulator)
        # aux: 2 banks (projections, Wo, transposes, FFN)
        SCP = ctx.enter_context(tc.tile_pool(name="scp", bufs=1, space="PSUM"))
        AVP = ctx.enter_context(tc.tile_pool(name="avp", bufs=1, space="PSUM"))
        AUX = ctx.enter_context(tc.tile_pool(name="aux", bufs=2, space="PSUM"))

        def aux_tile(shape, name):
            return AUX.tile(shape, F32, name=name, tag="aux", padded_shape=[128, 512])

        # ---- persistent SBUF ----
        P = ctx.enter_context(tc.tile_pool(name="persist", bufs=1))
        EP = ctx.enter_context(tc.tile_pool(name="epool", bufs=3))

        xq_sb = P.tile([C1, NQ], F32R, name="xq_sb")
        wkt_sb = [P.tile([128, D], F32R, name=f"wkt{c}") for c in range(2)]
        wvt_sb = [P.tile([128, D], F32R, name=f"wvt{c}") for c in range(2)]
        wqt_sb = P.tile([C1, D], F32R, name="wqt_sb")
        wrt_sb = P.tile([C1, D], F32R, name="wrt_sb")
        wot_sb = P.tile([128, 4, D], F32R, name="wot_sb")
        xc_sb = [P.tile([128, N_TOK], F32R, name=f"xc{c}") for c in range(2)]
        kt_sb = [P.tile([128, N_TOK], F16, name=f"kt{g}") for g in range(2)]
        # v64: per (kc, head) a [128k, 64] stationary = [V_h (32) | ones (32)]
        # so one matmul yields both attn-V and the softmax denominator.
        v64_sb = P.tile([128, KC, NH, 64], F16, name="v64_sb")
        qt_sb = [P.tile([128, NQ], F16, name=f"qt{g}") for g in range(2)]
        resid_sb = P.tile([128, QT_TILES, D], F32, name="resid_sb")
        # sparse attn layout, one tile per (hg, bank): rows 0-31 head 2b,
        # 32-63 its denominator, 64-95 head 2b+1, 96-127 its denominator
        attn_sp = [P.tile([128, NQ], F32R, name=f"attnsp{k}") for k in range(4)]
        s_av = [P.tile([128, 512], F32, name=f"s_av{b}") for b in range(2)]
        rec_st = [P.tile([128, 512], F32, name=f"rec_st{b}") for b in range(2)]
        w1t_sb = [P.tile([128, F1], F32R, name=f"w1t{c}") for c in range(2)]
        w2t_sb = P.tile([128, 8, D], F16, name="w2t_sb")
        x1_sb = P.tile([128, QT_TILES, D], F32, name="x1_sb")
        x1t_sb = [P.tile([128, NQ], F32R, name=f"x1t{g}") for g in range(2)]
        hdn_sb = P.tile([128, 8, NQ], F16, name="hdn_sb")
        bf1_col = P.tile([128, 8], F32, name="bf1_col")
        xwork = P.tile([128, 4, D], F32, name="xwork")
        xwork2 = P.tile([128, 4, D], F32, name="xwork2")
        mvb1 = P.tile([128, 4, 2], F32, name="mvb1")
        mvb2 = P.tile([128, 4, 2], F32, name="mvb2")
        rs1 = P.tile([128, 4], F32, name="rs1")
        rs2 = P.tile([128, 4], F32, name="rs2")
        lv1 = P.tile([128, 4], F32, name="lv1")
        lv2 = P.tile([128, 4], F32, name="lv2")
        ident = P.tile([128, 128], F32, name="ident")
        eps_sb = P.tile([128, 1], F32, name="eps_sb")
        dummy_sb = P.tile([128, 1], F32, name="dummy_sb")
        g1_bc = P.tile([128, D], F32, name="g1_bc")
        b1_bc = P.tile([128, D], F32, name="b1_bc")
        g2_bc = P.tile([128, D], F32, name="g2_bc")
        b2_bc = P.tile([128, D], F32, name="b2_bc")
        bf2_bc = P.tile([128, D], F32, name="bf2_bc")
        LNP = ctx.enter_context(tc.tile_pool(name="lnp", bufs=4))

        nc.vector.memset(v64_sb[:, :, :, 32:64], 1.0)
        nc.vector.memset(rec_st[0], 1.0)
        nc.vector.memset(rec_st[1], 1.0)
        nc.vector.memset(eps_sb, EPS)
        make_identity(nc, ident)
        # Pin the natural_log_exp_and_others ACT table set (only set with ln)
        # so the later LN Log calls never force a mid-kernel table reload.
        nc.scalar.activation(dummy_sb, eps_sb, AF.Ln, bias=eps_sb)

        def bcast_row(dst, src_ap, n):
            # (n,) dram -> (128, n) sbuf, replicated on all partitions
            import concourse.bass as bass
            src = bass.AP(tensor=src_ap.tensor, offset=src_ap.offset,
                          ap=[[0, 128]] + src_ap.ap)
            nc.sync.dma_start(dst, src)

        bcast_row(g1_bc, g1, D)
        bcast_row(b1_bc, b1, D)
        bcast_row(g2_bc, g2, D)
        bcast_row(b2_bc, b2, D)
        bcast_row(bf2_bc, bf2, D)

        nc.sync.dma_start(xq_sb, xq)
        nc.sync.dma_start(wot_sb, wot.rearrange("k p d -> p k d"))
        for c in range(2):
            nc.sync.dma_start(wkt_sb[c], wkt[128 * c:128 * (c + 1), :])
            nc.sync.dma_start(wvt_sb[c], wvt[128 * c:128 * (c + 1), :])
            nc.sync.dma_start(w1t_sb[c], w1t[128 * c:128 * (c + 1), :])
        nc.sync.dma_start(wqt_sb, wqt)
        nc.sync.dma_start(wrt_sb, wrt)
        nc.sync.dma_start(w2t_sb, w2t.rearrange("(a p) d -> p a d", p=128))
        nc.sync.dma_start(bf1_col, bf1.rearrange("(a p) -> p a", p=128))
        # xc in 512-column chunks so projection matmuls can chase the DMA
        for j in range(8):
            js = slice(512 * j, 512 * (j + 1))
            for c in range(2):
                nc.sync.dma_start(xc_sb[c][:, js], xc[128 * c:128 * (c + 1), js])

        # ---------- phase A building blocks (injected into sweeps) ----------
        def make_kt(hg, j):
            def f():
                js = slice(512 * j, 512 * (j + 1))
                kp = aux_tile([128, 512], "kp")
                for c in range(2):
                    nc.tensor.matmul(kp, wkt_sb[c][:, 128 * hg:128 * (hg + 1)],
                                     xc_sb[c][:, js],
                                     start=(c == 0), stop=(c == 1))
                nc.vector.tensor_copy(kt_sb[hg][:, js], kp)
            return f

        def make_v(kc):
            def f():
                ks = slice(128 * kc, 128 * (kc + 1))
                vp = aux_tile([128, D], "vp")
                for c in range(2):
                    nc.tensor.matmul(vp, xc_sb[c][:, ks], wvt_sb[c],
                                     start=(c == 0), stop=(c == 1))
                nc.vector.tensor_copy(
                    v64_sb[:, kc, :, 0:32],
                    vp.rearrange("p (h e) -> p h e", h=NH))
            return f

        def make_qt(hg, qc):
            def f():
                qs = slice(512 * qc, 512 * (qc + 1))
                qp = aux_tile([128, 512], "qp")
                nc.tensor.matmul(qp, wqt_sb[:, 128 * hg:128 * (hg + 1)],
                                 xq_sb[:, qs], start=True, stop=True)
                nc.vector.tensor_copy(qt_sb[hg][:, qs], qp)
            return f

        def make_resid(t):
            def f():
                rp = aux_tile([128, D], "rp")
                nc.tensor.matmul(rp, xq_sb[:, 128 * t:128 * (t + 1)], wrt_sb,
                                 start=True, stop=True)
                nc.vector.tensor_copy(resid_sb[:, t, :], rp)
            return f

        # ---------- phase C/D building blocks (Wo + LN1 + FFN + LN2) ----------
        def make_wo_ln1(qc, t):
            # t: local q-tile 0..3 within qc; global tile T = 4*qc + t
            def f():
                T = 4 * qc + t
                ts = slice(128 * T, 128 * (T + 1))
                pp = aux_tile([128, D], "pp")
                for k in range(4):
                    nc.tensor.matmul(pp, attn_sp[k][:, ts], wot_sb[:, k, :],
                                     start=(k == 0), stop=(k == 3))
                xp = xwork[:, t, :]
                nc.vector.tensor_add(xp, pp, resid_sb[:, T, :])
                stats = LNP.tile([128, 6], F32, name="stats", tag="st")
                nc.vector.bn_stats(out=stats, in_=xp)
                nc.vector.bn_aggr(out=mvb1[:, t, :], in_=stats)
            return f

        def make_rstd1():
            def f():
                nc.scalar.activation(lv1, mvb1[:, :, 1], AF.Ln, bias=eps_sb)
                nc.scalar.activation(rs1, lv1, AF.Exp, scale=-0.5)
            return f

        def make_x1(qc, t):
            def f():
                T = 4 * qc + t
                x1s = x1_sb[:, T, :]
                nc.vector.tensor_scalar(
                    out=x1s, in0=xwork[:, t, :], scalar1=mvb1[:, t, 0:1],
                    scalar2=rs1[:, t:t + 1], op0=OP.subtract, op1=OP.mult)
                nc.vector.tensor_mul(x1s, x1s, g1_bc)
                nc.vector.tensor_add(x1s, x1s, b1_bc)
            return f

        def make_x1t(qc, t):
            def f():
                T = 4 * qc + t
                ts = slice(128 * T, 128 * (T + 1))
                for dc in range(2):
                    tp = aux_tile([128, 128], "tp")
                    nc.tensor.transpose(
                        tp, x1_sb[:, T, 128 * dc:128 * (dc + 1)], ident)
                    nc.vector.tensor_copy(x1t_sb[dc][:, ts], tp)
            return f

        def make_ffn1(qc, fc):
            def f():
                qs = slice(512 * qc, 512 * (qc + 1))
                hp_ = aux_tile([128, 512], "hp_")
                for dc in range(2):
                    nc.tensor.matmul(
                        hp_, w1t_sb[dc][:, 128 * fc:128 * (fc + 1)],
                        x1t_sb[dc][:, qs], start=(dc == 0), stop=(dc == 1))
                nc.vector.tensor_scalar(
                    out=hdn_sb[:, fc, qs], in0=hp_,
                    scalar1=bf1_col[:, fc:fc + 1], scalar2=0.0,
                    op0=OP.add, op1=OP.max)
            return f

        def make_ffn2_ln2(qc, t):
            def f():
                T = 4 * qc + t
                ts = slice(128 * T, 128 * (T + 1))
                fp = aux_tile([128, D], "fp")
                for fc in range(8):
                    nc.tensor.matmul(fp, hdn_sb[:, fc, ts], w2t_sb[:, fc, :],
                                     start=(fc == 0), stop=(fc == 7))
                xp2 = xwork2[:, t, :]
                nc.vector.tensor_add(xp2, fp, x1_sb[:, T, :])
                nc.vector.tensor_add(xp2, xp2, bf2_bc)
                stats2 = LNP.tile([128, 6], F32, name="stats2", tag="st")
                nc.vector.bn_stats(out=stats2, in_=xp2)
                nc.vector.bn_aggr(out=mvb2[:, t, :], in_=stats2)
            return f

        def make_rstd2():
            def f():
                nc.scalar.activation(lv2, mvb2[:, :, 1], AF.Ln, bias=eps_sb)
                nc.scalar.activation(rs2, lv2, AF.Exp, scale=-0.5)
            return f

        def make_out(qc, t):
            def f():
                T = 4 * qc + t
                ts = slice(128 * T, 128 * (T + 1))
                xo = LNP.tile([128, D], F32, name="xo", tag="xo", bufs=2)
                nc.vector.tensor_scalar(
                    out=xo, in0=xwork2[:, t, :], scalar1=mvb2[:, t, 0:1],
                    scalar2=rs2[:, t:t + 1], op0=OP.subtract, op1=OP.mult)
                nc.vector.tensor_mul(xo, xo, g2_bc)
                nc.vector.tensor_add(xo, xo, b2_bc)
                nc.sync.dma_start(out[ts, :], xo)
            return f

        def cd_thunks(qc, part):
            """part 0: Wo+LN1+transpose+FFN1; part 1: FFN2+LN2+store."""
            th = []
            if part == 0:
                for t in range(4):
                    th.append(make_wo_ln1(qc, t))
                th.append(make_rstd1())
                for t in range(4):
                    th.append(make_x1(qc, t))
                    th.append(make_x1t(qc, t))
                for fc in range(8):
                    th.append(make_ffn1(qc, fc))
            else:
                for t in range(4):
                    th.append(make_ffn2_ln2(qc, t))
                th.append(make_rstd2())
                for t in range(4):
                    th.append(make_out(qc, t))
            return th

        # ---------- attention sweep ----------
        def sweep(hg, qc, inject):
            qs = slice(512 * qc, 512 * (qc + 1))
            sc4 = SCP.tile([128, 4, 512], F32, name="sc4", tag="sc")
            av = [AVP.tile([128, 512], F32, name=f"av{b}", tag=f"av{b}")
                  for b in range(2)]
            budget = max(1, -(-len(inject) // KC))

            def scores(kc, hls):
                ks = slice(128 * kc, 128 * (kc + 1))
                for hl in hls:
                    p = 32 * hl
                    nc.tensor.matmul(
                        sc4[:, hl, :], kt_sb[hg][p:p + 32, ks],
                        qt_sb[hg][p:p + 32, qs],
                        start=True, stop=True, tile_position=(p, 0))

            def av64(kc, b, e_t):
                # heads 2b,2b+1 -> bank b; each 64-col tile = [attn | denom]
                st, sp = (kc == 0), (kc == KC - 1)
                for j in range(2):
                    h = 4 * hg + 2 * b + j
                    nc.tensor.matmul(
                        av[b][64 * j:64 * j + 64, :], v64_sb[:, kc, h, :],
                        e_t[:, 2 * b + j, :], start=st, stop=sp,
                        tile_position=(0, 64 * j), skip_group_check=True)

            scores(0, (0, 1))
            scores(0, (2, 3))
            for kc in range(KC):
                e_t = EP.tile([128, 4, 512], F16, name="e_t", tag="e")
                nc.scalar.activation(e_t[:, 0:2, :], sc4[:, 0:2, :],
                                     AF.Exp, scale=SCALE)
                if kc + 1 < KC:
                    scores(kc + 1, (0, 1))
                av64(kc, 0, e_t)
                nc.scalar.activation(e_t[:, 2:4, :], sc4[:, 2:4, :],
                                     AF.Exp, scale=SCALE)
                if kc + 1 < KC:
                    scores(kc + 1, (2, 3))
                av64(kc, 1, e_t)
                for _ in range(budget):
                    if inject:
                        inject.popleft()()
            while inject:
                inject.popleft()()
            # normalize per bank: evacuate, shift denom rows under their
            # head's attn rows, reciprocal, multiply -> sparse attn tile
            for b in range(2):
                nc.vector.tensor_copy(s_av[b], av[b])
                nc.sync.dma_start(rec_st[b][0:32, :], s_av[b][32:64, :])
                nc.sync.dma_start(rec_st[b][64:96, :], s_av[b][96:128, :])
                nc.vector.reciprocal(rec_st[b], rec_st[b])
                nc.vector.tensor_mul(attn_sp[2 * hg + b][:, qs],
                                     s_av[b], rec_st[b])

        # ---------- issue the four sweeps ----------
        q1 = deque()
        for j in range(2):
            make_kt(0, j)()
        make_qt(0, 0)()
        for kc in range(4):
            make_v(kc)()
        for j in range(2, 8):
            q1.append(make_kt(0, j))
            for kc in range(4 * j - 4, 4 * j):
                q1.append(make_v(kc))
        for kc in range(28, KC):
            q1.append(make_v(kc))
        for j in range(8):
            q1.append(make_kt(1, j))
        q1.append(make_qt(1, 0))
        for t in range(QT_TILES):
            q1.append(make_resid(t))
        sweep(0, 0, q1)

        q2 = deque([make_qt(0, 1), make_qt(1, 1)])
        sweep(1, 0, q2)

        sweep(0, 1, deque(cd_thunks(0, 0)))
        sweep(1, 1, deque(cd_thunks(0, 1)))

        for th in cd_thunks(1, 0):
            th()
        for th in cd_thunks(1, 1):
            th()

    nc.compile()
    return nc


def _get_nc():
    global _built
    if _built is None:
        _built = _build()
    return _built


def _make_wot_sp(Wo):
    """Wo^T rows permuted into the sparse attn PSUM layout: block k=2*hg+b
    holds head (4hg+2b) dims at rows 0-31 and head (4hg+2b+1) dims at rows
    64-95; denominator/garbage rows get zero weights."""
    WoT = np.ascontiguousarray(Wo.T, np.float32)  # [d, e]
    wot_sp = np.zeros((4, 128, D), np.float32)
    for k in range(4):
        hg, b = divmod(k, 2)
        h0 = 4 * hg + 2 * b
        wot_sp[k, 0:32, :] = WoT[32 * h0:32 * (h0 + 1), :]
        wot_sp[k, 64:96, :] = WoT[32 * (h0 + 1):32 * (h0 + 2), :]
    return wot_sp


def _make_in_maps(inputs):
    f32 = np.float32
    f16 = np.float16
    F_lidar = np.ascontiguousarray(inputs["F_lidar"], dtype=f32)
    F_cam = np.ascontiguousarray(inputs["F_cam"], dtype=f32)
    common = {
        "wkt": np.ascontiguousarray(np.asarray(inputs["Wk"], f32).T),
        "wvt": np.ascontiguousarray(np.asarray(inputs["Wv"], f32).T),
        "wqt": np.ascontiguousarray(np.asarray(inputs["Wq"], f32).T),
        "wrt": np.ascontiguousarray(np.asarray(inputs["Wres"], f32).T),
        "wot": _make_wot_sp(np.asarray(inputs["Wo"], f32)),
        "w1t": np.ascontiguousarray(np.asarray(inputs["W1"], f32).T),
        "w2t": np.ascontiguousarray(np.asarray(inputs["W2"], f32).T.astype(f16)),
        "g1": np.asarray(inputs["g1"], f32), "b1": np.asarray(inputs["b1"], f32),
        "g2": np.asarray(inputs["g2"], f32), "b2": np.asarray(inputs["b2"], f32),
        "bf1": np.asarray(inputs["bf1"], f32),
        "bf2": np.asarray(inputs["bf2"], f32),
    }
    in_maps = []
    for c in range(N_CORES):
        b, s = c // CORES_PER_B, (c % CORES_PER_B) * NQ
        m = dict(common)
        m["xq"] = np.ascontiguousarray(
            F_lidar[b].reshape(C1, N_TOK)[:, s:s + NQ])
        m["xc"] = np.ascontiguousarray(F_cam[b].reshape(C2, N_TOK))
        in_maps.append(m)
    return in_maps


def kernel(**inputs):
    from concourse.bass_utils import run_bass_kernel_spmd

    nc = _get_nc()
    in_maps = _make_in_maps(inputs)
    res = run_bass_kernel_spmd(nc, in_maps, list(range(N_CORES)))
    out = np.empty((B, D, N_TOK), dtype=np.float32)
    for c in range(N_CORES):
        b, s = c // CORES_PER_B, (c % CORES_PER_B) * NQ
        out[b, :, s:s + NQ] = res.results[c]["out"].T
    return out.reshape(B, D, H, W)


# revision 17
# speedup vs baseline: 1.3833x; 1.3833x over previous
"""Trainium2 Bass kernel for nn_CrossAttentionFusion.

Problem (hardcoded shapes): B=2, C1=64, C2=256, D=256, NH=8, HD=32, H=W=64,
n = H*W = 4096 tokens per batch image.

    xl = F_lidar tokens (B, n, C1); xc = F_cam tokens (B, n, C2)
    Q = xl@Wq^T, K = xc@Wk^T, V = xc@Wv^T  (per-head HD=32)
    attn = softmax(QK^T/sqrt(HD)); out = attn@V
    x = LN1(xl@Wres^T + out@Wo^T); x = LN2(x + FFN(x)); return (B, D, H, W)

Sharding: 8 cores, zero collectives. Core i handles batch b=i//4 and the
1024-token q-slice (i%4). K/V for the whole image are recomputed per core.

Design: the kernel is ACT-bound (33.5M softmax exps per core = ~270us on the
scalar engine at 1 elem/lane/cycle). Everything else is organized to hide
under that floor:
  - Scores K^T@Q in fp16 with 4-head row-packing (tile_position); each head's
    [128,512] score block lands in its own PSUM bank (4 banks total).
  - exp runs in two [128,1024] halves (heads 01 / heads 23); while ACT exps
    one half, the PE refills the other half with the next kc's scores
    (half-quad double buffering inside 4 banks).
  - AV and the all-ones denominator matmuls are fp16 4-head column-packed,
    accumulating over kc into one PSUM bank each. Denominator rows align
    with AV head rows, so normalization is one DVE divide per (hg,qc).
  - Four sweeps (hg x qc). Projections (K/V/Q/resid) are injected into sweep
    1's PE slack; Wo+LN1+FFN for q-half 0 inject into sweeps 3/4; only the
    last q-half's epilogue is exposed.
  - LN rstd = Exp(-0.5*Log(var+eps)): Log+Exp live in one ACT table set
    (natural_log_exp_and_others), so no 2.7us table reloads interleave with
    the softmax exps. A dummy Log at kernel start pins that set.
"""

from collections import deque

import numpy as np

B, C1, C2, D, NH, H, W = 2, 64, 256, 256, 8, 64, 64
HD = D // NH                 # 32
N_TOK = H * W                # 4096 tokens per image
N_CORES = 8
CORES_PER_B = N_CORES // B   # 4
NQ = N_TOK // CORES_PER_B    # 1024 q tokens per core
EPS = 1e-5
SCALE = HD ** -0.5
KC = N_TOK // 128            # 32 k-chunks
QT_TILES = NQ // 128         # 8 q-tiles of 128
F1 = 4 * D                   # 1024 FFN hidden

_built = None


def _build():
    from contextlib import ExitStack

    import concourse.mybir as mybir
    import concourse.tile as tile
    from concourse import bacc
    from concourse.masks import make_identity

    F32 = mybir.dt.float32
    F32R = mybir.dt.float32r
    F16 = mybir.dt.float16
    AF = mybir.ActivationFunctionType
    OP = mybir.AluOpType

    nc = bacc.Bacc(trn_type="TRN2", target_bir_lowering=False, debug=False,
                   num_devices=N_CORES)

    # ---- DRAM I/O ----
    xq = nc.dram_tensor("xq", [C1, NQ], F32R, kind="ExternalInput").ap()
    xc = nc.dram_tensor("xc", [C2, N_TOK], F32R, kind="ExternalInput").ap()
    wkt = nc.dram_tensor("wkt", [C2, D], F32R, kind="ExternalInput").ap()
    wvt = nc.dram_tensor("wvt", [C2, D], F32R, kind="ExternalInput").ap()
    wqt = nc.dram_tensor("wqt", [C1, D], F32R, kind="ExternalInput").ap()
    wrt = nc.dram_tensor("wrt", [C1, D], F32R, kind="ExternalInput").ap()
    # Wo^T rows permuted+zero-padded to match the sparse attn PSUM layout
    # (4 blocks of 128 virtual-d rows; see _make_in_maps)
    wot = nc.dram_tensor("wot", [4, 128, D], F32R, kind="ExternalInput").ap()
    w1t = nc.dram_tensor("w1t", [D, F1], F32R, kind="ExternalInput").ap()
    w2t = nc.dram_tensor("w2t", [F1, D], F16, kind="ExternalInput").ap()
    g1 = nc.dram_tensor("g1", [D], F32, kind="ExternalInput").ap()
    b1 = nc.dram_tensor("b1", [D], F32, kind="ExternalInput").ap()
    g2 = nc.dram_tensor("g2", [D], F32, kind="ExternalInput").ap()
    b2 = nc.dram_tensor("b2", [D], F32, kind="ExternalInput").ap()
    bf1 = nc.dram_tensor("bf1", [F1], F32, kind="ExternalInput").ap()
    bf2 = nc.dram_tensor("bf2", [D], F32, kind="ExternalInput").ap()
    out = nc.dram_tensor("out", [NQ, D], F32, kind="ExternalOutput").ap()

    with tile.TileContext(nc) as tc, ExitStack() as ctx:
        # ---- PSUM pools (8 banks total) ----
        # sc: 4 banks (scores, one per head of the active group)
        # avp: 2 banks (attn-V accumulator + ones/denominator accumulator)
        # aux: 2 banks (projections, Wo, transposes, FFN)
        SCP = ctx.enter_context(tc.tile_pool(name="scp", bufs=1, space="PSUM"))
        AVP = ctx.enter_context(tc.tile_pool(name="avp", bufs=1, space="PSUM"))
        AUX = ctx.enter_context(tc.tile_pool(name="aux", bufs=2, space="PSUM"))

        def aux_tile(shape, name):
            return AUX.tile(shape, F32, name=name, tag="aux", padded_shape=[128, 512])

        # ---- persistent SBUF ----
        P = ctx.enter_context(tc.tile_pool(name="persist", bufs=1))
        EP = ctx.enter_context(tc.tile_pool(name="epool", bufs=3))

        xq_sb = P.tile([C1, NQ], F32R, name="xq_sb")
        wkt_sb = [P.tile([128, D], F32R, name=f"wkt{c}") for c in range(2)]
        wvt_sb = [P.tile([128, D], F32R, name=f"wvt{c}") for c in range(2)]
        wqt_sb = P.tile([C1, D], F32R, name="wqt_sb")
        wrt_sb = P.tile([C1, D], F32R, name="wrt_sb")
        wot_sb = P.tile([128, 4, D], F32R, name="wot_sb")
        xc_sb = [P.tile([128, N_TOK], F32R, name=f"xc{c}") for c in range(2)]
        kt_sb = [P.tile([128, N_TOK], F16, name=f"kt{g}") for g in range(2)]
        # v64: per (kc, head) a [128k, 64] stationary = [V_h (32) | ones (32)]
        # so one matmul yields both attn-V and the softmax denominator.
        v64_sb = P.tile([128, KC, NH, 64], F16, name="v64_sb")
        qt_sb = [P.tile([128, NQ], F16, name=f"qt{g}") for g in range(2)]
        resid_sb = P.tile([128, QT_TILES, D], F32, name="resid_sb")
        # sparse attn layout, one tile per (hg, bank): rows 0-31 head 2b,
        # 32-63 its denominator, 64-95 head 2b+1, 96-127 its denominator
        attn_sp = [P.tile([128, NQ], F32R, name=f"attnsp{k}") for k in range(4)]
        s_av = [P.tile([128, 512], F32, name=f"s_av{b}") for b in range(2)]
        rec_st = [P.tile([128, 512], F32, name=f"rec_st{b}") for b in range(2)]
        w1t_sb = [P.tile([128, F1], F32R, name=f"w1t{c}") for c in range(2)]
        w2t_sb = P.tile([128, 8, D], F16, name="w2t_sb")
        x1_sb = P.tile([128, QT_TILES, D], F32, name="x1_sb")
        x1t_sb = [P.tile([128, NQ], F32R, name=f"x1t{g}") for g in range(2)]
        hdn_sb = P.tile([128, 8, NQ], F16, name="hdn_sb")
        bf1_col = P.tile([128, 8], F32, name="bf1_col")
        xwork = P.tile([128, 4, D], F32, name="xwork")
        xwork2 = P.tile([128, 4, D], F32, name="xwork2")
        mvb1 = P.tile([128, 4, 2], F32, name="mvb1")
        mvb2 = P.tile([128, 4, 2], F32, name="mvb2")
        rs1 = P.tile([128, 4], F32, name="rs1")
        rs2 = P.tile([128, 4], F32, name="rs2")
        lv1 = P.tile([128, 4], F32, name="lv1")
        lv2 = P.tile([128, 4], F32, name="lv2")
        ident = P.tile([128, 128], F32, name="ident")
        eps_sb = P.tile([128, 1], F32, name="eps_sb")
        dummy_sb = P.tile([128, 1], F32, name="dummy_sb")
        g1_bc = P.tile([128, D], F32, name="g1_bc")
        b1_bc = P.tile([128, D], F32, name="b1_bc")
        g2_bc = P.tile([128, D], F32, name="g2_bc")
        b2_bc = P.tile([128, D], F32, name="b2_bc")
        bf2_bc = P.tile([128, D], F32, name="bf2_bc")
        LNP = ctx.enter_context(tc.tile_pool(name="lnp", bufs=4))

        nc.vector.memset(v64_sb[:, :, :, 32:64], 1.0)
        nc.vector.memset(rec_st[0], 1.0)
        nc.vector.memset(rec_st[1], 1.0)
        nc.vector.memset(eps_sb, EPS)
        make_identity(nc, ident)
        # Pin the natural_log_exp_and_others ACT table set (only set with ln)
        # so the later LN Log calls never force a mid-kernel table reload.
        nc.scalar.activation(dummy_sb, eps_sb, AF.Ln, bias=eps_sb)

        def bcast_row(dst, src_ap, n):
            # (n,) dram -> (128, n) sbuf, replicated on all partitions
            import concourse.bass as bass
            src = bass.AP(tensor=src_ap.tensor, offset=src_ap.offset,
                          ap=[[0, 128]] + src_ap.ap)
            nc.sync.dma_start(dst, src)

        bcast_row(g1_bc, g1, D)
        bcast_row(b1_bc, b1, D)
        bcast_row(g2_bc, g2, D)
        bcast_row(b2_bc, b2, D)
        bcast_row(bf2_bc, bf2, D)

        nc.sync.dma_start(xq_sb, xq)
        nc.sync.dma_start(wot_sb, wot.rearrange("k p d -> p k d"))
        for c in range(2):
            nc.sync.dma_start(wkt_sb[c], wkt[128 * c:128 * (c + 1), :])
            nc.sync.dma_start(wvt_sb[c], wvt[128 * c:128 * (c + 1), :])
            nc.sync.dma_start(w1t_sb[c], w1t[128 * c:128 * (c + 1), :])
        nc.sync.dma_start(wqt_sb, wqt)
        nc.sync.dma_start(wrt_sb, wrt)
        nc.sync.dma_start(w2t_sb, w2t.rearrange("(a p) d -> p a d", p=128))
        nc.sync.dma_start(bf1_col, bf1.rearrange("(a p) -> p a", p=128))
        # xc in 512-column chunks so projection matmuls can chase the DMA
        for j in range(8):
            js = slice(512 * j, 512 * (j + 1))
            for c in range(2):
                nc.sync.dma_start(xc_sb[c][:, js], xc[128 * c:128 * (c + 1), js])

        # ---------- phase A building blocks (injected into sweeps) ----------
        def make_kt(hg, j):
            def f():
                js = slice(512 * j, 512 * (j + 1))
                kp = aux_tile([128, 512], "kp")
                for c in range(2):
                    nc.tensor.matmul(kp, wkt_sb[c][:, 128 * hg:128 * (hg + 1)],
                                     xc_sb[c][:, js],
                                     start=(c == 0), stop=(c == 1))
                nc.vector.tensor_copy(kt_sb[hg][:, js], kp)
            return f

        def make_v(kc):
            def f():
                ks = slice(128 * kc, 128 * (kc + 1))
                vp = aux_tile([128, D], "vp")
                for c in range(2):
                    nc.tensor.matmul(vp, xc_sb[c][:, ks], wvt_sb[c],
                                     start=(c == 0), stop=(c == 1))
                nc.vector.tensor_copy(
                    v64_sb[:, kc, :, 0:32],
                    vp.rearrange("p (h e) -> p h e", h=NH))
            return f

        def make_qt(hg, qc):
            def f():
                qs = slice(512 * qc, 512 * (qc + 1))
                qp = aux_tile([128, 512], "qp")
                nc.tensor.matmul(qp, wqt_sb[:, 128 * hg:128 * (hg + 1)],
                                 xq_sb[:, qs], start=True, stop=True)
                nc.vector.tensor_copy(qt_sb[hg][:, qs], qp)
            return f

        def make_resid(t):
            def f():
                rp = aux_tile([128, D], "rp")
                nc.tensor.matmul(rp, xq_sb[:, 128 * t:128 * (t + 1)], wrt_sb,
                                 start=True, stop=True)
                nc.vector.tensor_copy(resid_sb[:, t, :], rp)
            return f

        # ---------- phase C/D building blocks (Wo + LN1 + FFN + LN2) ----------
        def make_wo_ln1(qc, t):
            # t: local q-tile 0..3 within qc; global tile T = 4*qc + t
            def f():
                T = 4 * qc + t
                ts = slice(128 * T, 128 * (T + 1))
                pp = aux_tile([128, D], "pp")
                for k in range(4):
                    nc.tensor.matmul(pp, attn_sp[k][:, ts], wot_sb[:, k, :],
                                     start=(k == 0), stop=(k == 3))
                xp = xwork[:, t, :]
                nc.vector.tensor_add(xp, pp, resid_sb[:, T, :])
                stats = LNP.tile([128, 6], F32, name="stats", tag="st")
                nc.vector.bn_stats(out=stats, in_=xp)
                nc.vector.bn_aggr(out=mvb1[:, t, :], in_=stats)
            return f

        def make_rstd1():
            def f():
                nc.scalar.activation(lv1, mvb1[:, :, 1], AF.Ln, bias=eps_sb)
                nc.scalar.activation(rs1, lv1, AF.Exp, scale=-0.5)
            return f

        def make_x1(qc, t):
            def f():
                T = 4 * qc + t
                x1s = x1_sb[:, T, :]
                nc.vector.tensor_scalar(
                    out=x1s, in0=xwork[:, t, :], scalar1=mvb1[:, t, 0:1],
                    scalar2=rs1[:, t:t + 1], op0=OP.subtract, op1=OP.mult)
                nc.vector.tensor_mul(x1s, x1s, g1_bc)
                nc.vector.tensor_add(x1s, x1s, b1_bc)
            return f

        def make_x1t(qc, t):
            def f():
                T = 4 * qc + t
                ts = slice(128 * T, 128 * (T + 1))
                for dc in range(2):
                    tp = aux_tile([128, 128], "tp")
                    nc.tensor.transpose(
                        tp, x1_sb[:, T, 128 * dc:128 * (dc + 1)], ident)
                    nc.vector.tensor_copy(x1t_sb[dc][:, ts], tp)
            return f

        def make_ffn1(qc, fc):
            def f():
                qs = slice(512 * qc, 512 * (qc + 1))
                hp_ = aux_tile([128, 512], "hp_")
                for dc in range(2):
                    nc.tensor.matmul(
                        hp_, w1t_sb[dc][:, 128 * fc:128 * (fc + 1)],
                        x1t_sb[dc][:, qs], start=(dc == 0), stop=(dc == 1))
                nc.vector.tensor_scalar(
                    out=hdn_sb[:, fc, qs], in0=hp_,
                    scalar1=bf1_col[:, fc:fc + 1], scalar2=0.0,
                    op0=OP.add, op1=OP.max)
            return f

        def make_ffn2_ln2(qc, t):
            def f():
                T = 4 * qc + t
                ts = slice(128 * T, 128 * (T + 1))
                fp = aux_tile([128, D], "fp")
                for fc in range(8):
                    nc.tensor.matmul(fp, hdn_sb[:, fc, ts], w2t_sb[:, fc, :],
                                     start=(fc == 0), stop=(fc == 7))
                xp2 = xwork2[:, t, :]
                nc.vector.tensor_add(xp2, fp, x1_sb[:, T, :])
                nc.vector.tensor_add(xp2, xp2, bf2_bc)
                stats2 = LNP.tile([128, 6], F32, name="stats2", tag="st")
                nc.vector.bn_stats(out=stats2, in_=xp2)
                nc.vector.bn_aggr(out=mvb2[:, t, :], in_=stats2)
            return f

        def make_rstd2():
            def f():
                nc.scalar.activation(lv2, mvb2[:, :, 1], AF.Ln, bias=eps_sb)
                nc.scalar.activation(rs2, lv2, AF.Exp, scale=-0.5)
            return f

        def make_out(qc, t):
            def f():
                T = 4 * qc + t
                ts = slice(128 * T, 128 * (T + 1))
                xo = LNP.tile([128, D], F32, name="xo", tag="xo", bufs=2)
                nc.vector.tensor_scalar(
                    out=xo, in0=xwork2[:, t, :], scalar1=mvb2[:, t, 0:1],
                    scalar2=rs2[:, t:t + 1], op0=OP.subtract, op1=OP.mult)
                nc.vector.tensor_mul(xo, xo, g2_bc)
                nc.vector.tensor_add(xo, xo, b2_bc)
                nc.sync.dma_start(out[ts, :], xo)
            return f

        def cd_thunks(qc, part):
            """part 0: Wo+LN1+transpose+FFN1; part 1: FFN2+LN2+store."""
            th = []
            if part == 0:
                for t in range(4):
                    th.append(make_wo_ln1(qc, t))
                th.append(make_rstd1())
                for t in range(4):
                    th.append(make_x1(qc, t))
                    th.append(make_x1t(qc, t))
                for fc in range(8):
                    th.append(make_ffn1(qc, fc))
            else:
                for t in range(4):
                    th.append(make_ffn2_ln2(qc, t))
                th.append(make_rstd2())
                for t in range(4):
                    th.append(make_out(qc, t))
            return th

        # ---------- attention sweep ----------
        def sweep(hg, qc, inject):
            qs = slice(512 * qc, 512 * (qc + 1))
            # separate PSUM/SBUF tiles per half so the dependency tracker
            # never chains E_b behind S01(kc+1) (whole-tile RAW hazard)
            sc = [SCP.tile([128, 2, 512], F32, name=f"sc{h}", tag=f"sc{h}")
                  for h in range(2)]
            av = [AVP.tile([128, 512], F32, name=f"av{b}", tag=f"av{b}")
                  for b in range(2)]
            budget = max(1, -(-len(inject) // KC))

            def scores(kc, b):
                ks = slice(128 * kc, 128 * (kc + 1))
                for j in range(2):
                    p = 32 * (2 * b + j)
                    nc.tensor.matmul(
                        sc[b][:, j, :], kt_sb[hg][p:p + 32, ks],
                        qt_sb[hg][p:p + 32, qs],
                        start=True, stop=True, tile_position=(p, 0))

            def av64(kc, b, e_t):
                # heads 2b,2b+1 -> bank b; each 64-col tile = [attn | denom]
                st, sp = (kc == 0), (kc == KC - 1)
                for j in range(2):
                    h = 4 * hg + 2 * b + j
                    nc.tensor.matmul(
                        av[b][64 * j:64 * j + 64, :], v64_sb[:, kc, h, :],
                        e_t[:, j, :], start=st, stop=sp,
                        tile_position=(0, 64 * j), skip_group_check=True)

            scores(0, 0)
            scores(0, 1)
            for kc in range(KC):
                e_a = EP.tile([128, 2, 512], F16, name="e_a", tag="ea")
                e_b = EP.tile([128, 2, 512], F16, name="e_b", tag="eb")
                nc.scalar.activation(e_a, sc[0], AF.Exp, scale=SCALE)
                if kc + 1 < KC:
                    scores(kc + 1, 0)
                av64(kc, 0, e_a)
                nc.scalar.activation(e_b, sc[1], AF.Exp, scale=SCALE)
                if kc + 1 < KC:
                    scores(kc + 1, 1)
                av64(kc, 1, e_b)
                for _ in range(budget):
                    if inject:
                        inject.popleft()()
            while inject:
                inject.popleft()()
            # normalize per bank: evacuate, shift denom rows under their
            # head's attn rows, reciprocal, multiply -> sparse attn tile
            for b in range(2):
                nc.vector.tensor_copy(s_av[b], av[b])
                nc.sync.dma_start(rec_st[b][0:32, :], s_av[b][32:64, :])
                nc.sync.dma_start(rec_st[b][64:96, :], s_av[b][96:128, :])
                nc.vector.reciprocal(rec_st[b], rec_st[b])
                nc.vector.tensor_mul(attn_sp[2 * hg + b][:, qs],
                                     s_av[b], rec_st[b])

        # ---------- issue the four sweeps ----------
        q1 = deque()
        for j in range(2):
            make_kt(0, j)()
        make_qt(0, 0)()
        for kc in range(4):
            make_v(kc)()
        for j in range(2, 8):
            q1.append(make_kt(0, j))
            for kc in range(4 * j - 4, 4 * j):
                q1.append(make_v(kc))
        for kc in range(28, KC):
            q1.append(make_v(kc))
        for j in range(8):
            q1.append(make_kt(1, j))
        q1.append(make_qt(1, 0))
        for t in range(QT_TILES):
            q1.append(make_resid(t))
        sweep(0, 0, q1)

        q2 = deque([make_qt(0, 1), make_qt(1, 1)])
        sweep(1, 0, q2)

        sweep(0, 1, deque(cd_thunks(0, 0)))
        sweep(1, 1, deque(cd_thunks(0, 1)))

        for th in cd_thunks(1, 0):
            th()
        for th in cd_thunks(1, 1):
            th()

    nc.compile()
    return nc


def _get_nc():
    global _built
    if _built is None:
        _built = _build()
    return _built


def _make_wot_sp(Wo):
    """Wo^T rows permuted into the sparse attn PSUM layout: block k=2*hg+b
    holds head (4hg+2b) dims at rows 0-31 and head (4hg+2b+1) dims at rows
    64-95; denominator/garbage rows get zero weights."""
    WoT = np.ascontiguousarray(Wo.T, np.float32)  # [d, e]
    wot_sp = np.zeros((4, 128, D), np.float32)
    for k in range(4):
        hg, b = divmod(k, 2)
        h0 = 4 * hg + 2 * b
        wot_sp[k, 0:32, :] = WoT[32 * h0:32 * (h0 + 1), :]
        wot_sp[k, 64:96, :] = WoT[32 * (h0 + 1):32 * (h0 + 2), :]
    return wot_sp


def _make_in_maps(inputs):
    f32 = np.float32
    f16 = np.float16
    F_lidar = np.ascontiguousarray(inputs["F_lidar"], dtype=f32)
    F_cam = np.ascontiguousarray(inputs["F_cam"], dtype=f32)
    common = {
        "wkt": np.ascontiguousarray(np.asarray(inputs["Wk"], f32).T),
        "wvt": np.ascontiguousarray(np.asarray(inputs["Wv"], f32).T),
        "wqt": np.ascontiguousarray(np.asarray(inputs["Wq"], f32).T),
        "wrt": np.ascontiguousarray(np.asarray(inputs["Wres"], f32).T),
        "wot": _make_wot_sp(np.asarray(inputs["Wo"], f32)),
        "w1t": np.ascontiguousarray(np.asarray(inputs["W1"], f32).T),
        "w2t": np.ascontiguousarray(np.asarray(inputs["W2"], f32).T.astype(f16)),
        "g1": np.asarray(inputs["g1"], f32), "b1": np.asarray(inputs["b1"], f32),
        "g2": np.asarray(inputs["g2"], f32), "b2": np.asarray(inputs["b2"], f32),
        "bf1": np.asarray(inputs["bf1"], f32),
        "bf2": np.asarray(inputs["bf2"], f32),
    }
    in_maps = []
    for c in range(N_CORES):
        b, s = c // CORES_PER_B, (c % CORES_PER_B) * NQ
        m = dict(common)
        m["xq"] = np.ascontiguousarray(
            F_lidar[b].reshape(C1, N_TOK)[:, s:s + NQ])
        m["xc"] = np.ascontiguousarray(F_cam[b].reshape(C2, N_TOK))
        in_maps.append(m)
    return in_maps


def kernel(**inputs):
    from concourse.bass_utils import run_bass_kernel_spmd

    nc = _get_nc()
    in_maps = _make_in_maps(inputs)
    res = run_bass_kernel_spmd(nc, in_maps, list(range(N_CORES)))
    out = np.empty((B, D, N_TOK), dtype=np.float32)
    for c in range(N_CORES):
        b, s = c // CORES_PER_B, (c % CORES_PER_B) * NQ
        out[b, :, s:s + NQ] = res.results[c]["out"].T
    return out.reshape(B, D, H, W)
